# revision 23
# baseline (speedup 1.0000x reference)
"""Trainium2 Bass kernel for nn_ExpDock (keypoint cross-attention + Kabsch).

Math (per complex b):
    h2bar = mean_m H2[b]                  -> v1_k = W1_k @ h2bar
    s1[k,n] = <H1[b,n], v1_k>/sqrt(d)     -> a1 = softmax_n(s1)
    Y1[k]  = sum_n a1[k,n] X1[b,n]        (and symmetrically Y2 from H2/X2)
    output = stack([Y1, Y2, kabsch(Y1, Y2)])

Distribution: pure data-parallel over B=16 complexes, 2 per NeuronCore.

Device-side design (per core, per complex, per side):
  - H fed host-transposed [d=128, N] in fp8-e4m3 so the feature axis sits on
    SBUF partitions AND the dominant HBM stream is 1 byte/elem: the kernel is
    memory-bound, so fp8 halves the roofline vs fp16 (~1e-3 score noise).
  - tiny v_k = W_k @ hbar vectors (0.4% of FLOPs) folded into host prep with
    the 1/(N*sqrt(d)) scale and a x64 power-of-two gain so fp8 stores them in
    normal range; the exp activation applies the 1/64 back on ACT for free.
  - scores for 4 groups of 1024 columns computed concurrently via PE
    column-tiling (tile_position=(0,32g)); one wide [128, 1024] exp per pass
    amortizes ACT's 352-cycle fixed cost.
  - p = exp(s) transposed back to m-on-partitions with PE matmuls whose
    moving operand is a host-built [128, 40] row-SELECTOR (not the identity):
    the transpose emits only the 40 useful (group, keypoint) rows, k-dense,
    cutting transpose streams, DVE copy width and Y-matmul width by 3.2x.
  - per 128-column block, one column-tiled matmul against a host-prebuilt
    block-diagonal X (+ones) tile accumulates Y numerator and softmax
    denominator into a [128, 40] PSUM accumulator (4 strips x 13 rows).
  - H streams as 1 MB chunks on TWO DMA queues (sync HWDGE + gpsimd SWDGE),
    one queue per side of an interleaved side-PAIR, so delivery matches the
    interleaved consumption order and neither queue ever starves compute:
    per-queue DMA rate is descriptor-latency-bound (~150 GB/s at 8 KB
    partition lines), so both queues must run concurrently; the scalar
    engine stays free to run only the exp activations.
  - host divides by Z and runs the 16 tiny 3x3 Kabsch SVDs.
"""

from contextlib import ExitStack

import numpy as np

import concourse.bass as bass
import concourse.tile as tile
from concourse import bacc
from concourse import masks, mybir
from concourse._compat import with_exitstack
from concourse.bass_utils import run_bass_kernel_spmd

B, N, D, K = 16, 16384, 128, 10
NCORES = 8
BPC = B // NCORES          # complexes per core
CH = 1024                  # score columns per group per pass
G = 4                      # PE column-tile groups (concurrent chunks)
PASS = CH * G              # m's consumed per group-pass
NPASS = N // PASS          # group-passes per (b, side)
J = CH // 128              # 128-column transpose blocks per pass
KK = G * K                 # packed keypoint columns after selector transpose
CPS = 2                    # DMA chunks per side (1 MB each, 2 passes)
FP = mybir.dt.float16
FP8 = mybir.dt.float8e4
F32 = mybir.dt.float32
VS = 64.0                  # v pre-gain so fp8(v) stays in normal range
SCALE = 1.0 / (N * np.sqrt(D))   # mean + 1/sqrt(d), folded into v on host
XBC = NPASS * J * 13       # xb columns per (b, side)


@with_exitstack
def _body(ctx, tc, hts, xbs, vv, sel_d, out):
    nc = tc.nc

    const = ctx.enter_context(tc.tile_pool(name="const", bufs=1))
    hstream = ctx.enter_context(tc.tile_pool(name="hstream", bufs=6))
    small = ctx.enter_context(tc.tile_pool(name="small", bufs=4))
    ppool = ctx.enter_context(tc.tile_pool(name="p", bufs=4))
    ptsb = ctx.enter_context(tc.tile_pool(name="ptsb", bufs=4))
    sps = ctx.enter_context(tc.tile_pool(name="spsum", bufs=2, space="PSUM"))
    ptps = ctx.enter_context(tc.tile_pool(name="ptpsum", bufs=2, space="PSUM"))
    ynps = ctx.enter_context(tc.tile_pool(name="ynum", bufs=2, space="PSUM"))

    ident = const.tile([128, 128], FP)
    masks.make_identity(nc, ident[:])
    junk = const.tile([128, 512], FP, tag="junk")
    nc.vector.memset(junk[:], 0.25)

    # Small constants: v + sel on the scalar HWDGE queue in first-need
    # order; the per-side X tiles ride the queue whose ring is free before
    # each side's first Y matmul needs them.
    v_all = const.tile([128, BPC * 2 * 32], FP8, tag="v")
    nc.scalar.dma_start(out=v_all[:], in_=vv[:, :])
    sel = const.tile([128, KK], FP, tag="sel")
    nc.scalar.dma_start(out=sel[:], in_=sel_d[:, :])
    xb_t = []
    for sg in range(4):
        xb1 = const.tile([128, XBC], FP, tag=f"xb{sg}", name=f"xb{sg}")
        xb_t.append(xb1)

    # H stream: sides are processed as interleaved PAIRS (A,B then C,D).
    # Early pair: small chunks (fast first-arrival) on the fast-starting
    # sync/scalar HWDGE rings; late pair: whole-side 2 MB chunks (16 KB
    # partition lines -> ~230 GB/s ring rate) on the slow-starting SWDGE
    # ring.  Scalar carries side B, but only its first chunk is issued here
    # -- the rest are injected into the compute loop at points where the
    # scalar ring is already free, so exp activations never queue behind a
    # ring-wait.  Every chunk gets its own SBUF slot (no WAR stalls).
    # Whole-side 2 MB chunks (16 KB partition lines -> max ring rate, and
    # both HWDGE+SWDGE rings then split HBM evenly) in strict consumption
    # order; C's first half rides the scalar ring early so the pair-1
    # ramp-up overlaps pair-0 compute.  The tiny xb tiles also crawl on
    # the scalar ring from t=0 -- DMA issue does not block the engine, so
    # the exp activations are unaffected.
    PLAN = {0: [2, 2], 1: [2, 2], 2: [2, 2], 3: [4]}
    h_tiles = {}

    def h_dma(q, sg, c):
        plan = PLAN[sg]
        p0 = sum(plan[:c])
        np_ = plan[c]
        b, side = sg // 2, sg % 2
        hc = hstream.tile([128, np_ * PASS], FP8, tag=f"hc{np_}",
                          bufs=8, name=f"hc{sg}_{c}")
        q.dma_start(out=hc[:],
                    in_=hts[side][b, :, p0 * PASS:(p0 + np_) * PASS])
        for pp in range(p0, p0 + np_):
            h_tiles[(sg, pp)] = (hc, (pp - p0) * PASS)

    nc.scalar.dma_start(out=xb_t[0][:], in_=xbs[:, 0 * XBC:1 * XBC])
    nc.scalar.dma_start(out=xb_t[1][:], in_=xbs[:, 1 * XBC:2 * XBC])
    h_dma(nc.sync, 0, 0)       # A first half: early pair in 1 MB halves
    h_dma(nc.gpsimd, 1, 0)     # B first half
    h_dma(nc.scalar, 2, 0)     # C first half (1 MB), early on scalar
    nc.scalar.dma_start(out=xb_t[2][:], in_=xbs[:, 2 * XBC:3 * XBC])
    nc.scalar.dma_start(out=xb_t[3][:], in_=xbs[:, 3 * XBC:4 * XBC])
    h_dma(nc.sync, 0, 1)       # A second half
    h_dma(nc.gpsimd, 1, 1)     # B second half
    h_dma(nc.sync, 2, 1)       # C second half (1 MB) after A
    h_dma(nc.gpsimd, 3, 0)     # D: one 2 MB chunk after B

    def do_scores(v, hc, off):
        """One pass of score matmuls + one wide exp; returns p_sb [128, CH]."""
        s_ps = sps.tile([128, CH], F32)
        for h in range(CH // 512):
            for g in range(G):   # h-outer: the 4 groups run concurrently
                nc.tensor.matmul(
                    s_ps[32 * g:32 * (g + 1), 512 * h:512 * (h + 1)],
                    v[:], hc[:, off + g * CH + 512 * h:
                             off + g * CH + 512 * (h + 1)],
                    start=True, stop=True, tile_position=(0, 32 * g))
        p_sb = ppool.tile([128, CH], FP)
        nc.scalar.activation(p_sb[:], s_ps[:],
                             mybir.ActivationFunctionType.Exp, scale=1.0 / VS)
        return p_sb

    def do_transp(p_sb):
        """Selector transposes: pt[f, KK*j + 10g+k] = p[32g+k, 128j+f]."""
        pt_ps = ptps.tile([128, J * KK], F32)
        for j in range(J):
            nc.tensor.matmul(pt_ps[:, KK * j:KK * (j + 1)],
                             p_sb[:, 128 * j:128 * (j + 1)],
                             sel[:], start=True, stop=True)
        pt = ptsb.tile([128, J * KK], FP)
        nc.vector.tensor_copy(pt[:], pt_ps[:])
        return pt

    def do_y(pt, p, yn, xb):
        # Column-tiled concurrent matmuls: block j accumulates into
        # partition strip 32*(j%4) (13 useful rows) of the [128, KK] acc.
        for j in range(J):
            nc.tensor.matmul(
                yn[32 * (j % 4):32 * (j % 4) + 13, :],
                xb[:, (p * J + j) * 13:(p * J + j + 1) * 13],
                pt[:, KK * j:KK * (j + 1)],
                start=(p == 0 and j < 4),
                stop=(p == NPASS - 1 and j >= J - 4),
                tile_position=(0, 32 * (j % 4)))

    # PE warm-up on the device-built identity (no DMA dependency): ~6us of
    # back-to-back matmuls so HAM un-throttles the PE clock (1.2 -> 2.4 GHz)
    # by the time the first H chunk lands.  The first real Y accumulation
    # overwrites this tile via its start=True flag.
    warm_ps = ynps.tile([128, 512], F32, tag="yn")
    for _ in range(45):
        nc.tensor.matmul(warm_ps[:], ident[:], junk[:], start=True, stop=True)

    def side_state(sg):
        b, side = sg // 2, sg % 2
        return {
            "v": v_all[:, sg * 32:sg * 32 + 32],
            "xb": xb_t[sg][:],
            "yn": ynps.tile([128, KK], F32, name=f"yn{sg}", tag="yn"),
            "pend_e": None, "pend_y": None, "sg": sg,
        }

    def step(st, p):
        """Advance one side's 2-deep pipeline by one pass (p=None: drain)."""
        if p is not None:
            hc, off = h_tiles[(st["sg"], p)]
            p_sb = do_scores(st["v"], hc, off)
        if st["pend_e"] is not None:
            pt = do_transp(st["pend_e"][0])
            if st["pend_y"] is not None:
                do_y(*st["pend_y"], st["yn"], st["xb"])
            st["pend_y"] = (pt, st["pend_e"][1])
            st["pend_e"] = None
        if p is not None:
            st["pend_e"] = (p_sb, p)
        elif st["pend_y"] is not None:
            do_y(*st["pend_y"], st["yn"], st["xb"])
            st["pend_y"] = None

    for pair in range(2):
        sts = [side_state(pair * 2), side_state(pair * 2 + 1)]
        for p in range(NPASS):
            for st in sts:
                step(st, p)
        for st in sts:          # drain: pending transpose, then pending Y
            step(st, None)
            step(st, None)
        for st in sts:
            yn_sb = small.tile([128, KK], F32, tag="yn_sb")
            nc.vector.tensor_copy(yn_sb[:], st["yn"][:])
            b, side = st["sg"] // 2, st["sg"] % 2
            nc.sync.dma_start(out=out[b, side], in_=yn_sb[:])


_NC_CACHE = {}


def _build_nc():
    if "nc" in _NC_CACHE:
        return _NC_CACHE["nc"]
    nc = bacc.Bacc(None)
    h1t = nc.declare_dram_parameter("h1t", [BPC, D, N], FP8, isOutput=False)
    h2t = nc.declare_dram_parameter("h2t", [BPC, D, N], FP8, isOutput=False)
    xb = nc.declare_dram_parameter("xb", [128, BPC * 2 * XBC], FP,
                                   isOutput=False)
    vv = nc.declare_dram_parameter("vv", [128, BPC * 2 * 32], FP8,
                                   isOutput=False)
    sel = nc.declare_dram_parameter("sel", [128, KK], FP, isOutput=False)
    out = nc.declare_dram_parameter("out", [BPC, 2, 128, KK], F32,
                                    isOutput=True)
    with tile.TileContext(nc) as tc:
        _body(tc, (h1t, h2t), xb, vv, sel, out)
    nc.compile()
    _NC_CACHE["nc"] = nc
    return nc


def _make_xblk(X):
    """X [B, N, 3] f32 -> [B, 128, XBC] fp16 block-diagonal layout.

    Column (p*J+j)*13 + 3g+c at partition f holds X[b, (p*G+g)*CH + j*128 + f, c];
    column (p*J+j)*13 + 12 is 1.0 (softmax-denominator ones column).
    """
    Bn = X.shape[0]
    Xr = X.reshape(Bn, NPASS, G, J, 128, 3).transpose(0, 1, 3, 4, 2, 5)
    Xb = Xr.reshape(Bn, NPASS, J, 128, 12)
    ones = np.ones((Bn, NPASS, J, 128, 1), np.float32)
    full = np.concatenate([Xb, ones], -1)            # [B, NPASS, J, 128, 13]
    return np.ascontiguousarray(
        full.transpose(0, 3, 1, 2, 4).reshape(Bn, 128, XBC)
    ).astype(np.float16)


def _prep(H1, H2, X1, X2, W1, W2):
    f8 = mybir.dt.np(FP8)
    h1T = np.ascontiguousarray(H1.transpose(0, 2, 1)).astype(f8)
    h2T = np.ascontiguousarray(H2.transpose(0, 2, 1)).astype(f8)
    xb = np.stack([_make_xblk(X1), _make_xblk(X2)], axis=1)  # [B,2,128,XBC]
    # v[b, 0] = W1 @ scaled-mean(H2[b]) drives the H1-side scores;
    # v[b, 1] = W2 @ scaled-mean(H1[b]).  x64 keeps fp8 in normal range;
    # the exp activation divides it back out.
    vv = np.zeros((B, 2, 128, 32), np.float32)
    vv[:, 0, :, :K] = np.einsum('kde,be->bdk', W1, H2.sum(1)) * (SCALE * VS)
    vv[:, 1, :, :K] = np.einsum('kde,be->bdk', W2, H1.sum(1)) * (SCALE * VS)
    vv = vv.astype(f8)
    # Row selector: sel[32g+k, 10g+k] = 1 picks the 40 useful score rows.
    sel = np.zeros((128, KK), np.float16)
    for g in range(G):
        for k in range(K):
            sel[32 * g + k, K * g + k] = 1.0
    in_maps = []
    for c in range(NCORES):
        s = slice(c * BPC, (c + 1) * BPC)
        xbc = xb[s].transpose(2, 0, 1, 3).reshape(128, -1)
        vvc = vv[s].transpose(2, 0, 1, 3).reshape(128, -1)
        in_maps.append({
            "h1t": h1T[s], "h2t": h2T[s],
            "xb": np.ascontiguousarray(xbc),
            "vv": np.ascontiguousarray(vvc),
            "sel": sel,
        })
    return in_maps


def _kabsch_np(P, Q):
    c1 = P.mean(0)
    c2 = Q.mean(0)
    Hm = (P - c1).T @ (Q - c2)
    U, _, Vt = np.linalg.svd(Hm)
    sign = np.sign(np.linalg.det(U @ Vt))
    R = U @ np.diag([1.0, 1.0, sign]) @ Vt
    t = c2 - c1 @ R
    return P @ R + t


def _finalize(res):
    Y = np.zeros((B, 2, K, 3), np.float32)
    for c in range(NCORES):
        yn = np.asarray(res[c]["out"], np.float32)  # [BPC, 2, 128, KK]
        for bl in range(BPC):
            for side in range(2):
                acc = yn[bl, side]
                Ynum = np.zeros((K, 3), np.float32)
                Z = np.zeros(K, np.float32)
                for s in range(4):
                    for g in range(G):
                        Z += acc[32 * s + 12, K * g:K * g + K]
                        Ynum += acc[32 * s + 3 * g:32 * s + 3 * g + 3,
                                    K * g:K * g + K].T
                Y[c * BPC + bl, side] = Ynum / Z[:, None]
    Y1, Y2 = Y[:, 0], Y[:, 1]
    Y1a = np.stack([
        _kabsch_np(Y1[b].astype(np.float64), Y2[b].astype(np.float64))
        for b in range(B)
    ]).astype(np.float32)
    return np.stack([Y1, Y2, Y1a], axis=1)


def kernel(H1, H2, X1, X2, W1, W2):
    args = [np.asarray(a, np.float32) for a in (H1, H2, X1, X2, W1, W2)]
    in_maps = _prep(*args)
    nc = _build_nc()
    res = run_bass_kernel_spmd(nc, in_maps, list(range(NCORES))).results
    return _finalize(res)


# revision 24
# speedup vs baseline: 1.0368x; 1.0368x over previous
"""Trainium2 Bass kernel for nn_ExpDock (keypoint cross-attention + Kabsch).

Math (per complex b):
    h2bar = mean_m H2[b]                  -> v1_k = W1_k @ h2bar
    s1[k,n] = <H1[b,n], v1_k>/sqrt(d)     -> a1 = softmax_n(s1)
    Y1[k]  = sum_n a1[k,n] X1[b,n]        (and symmetrically Y2 from H2/X2)
    output = stack([Y1, Y2, kabsch(Y1, Y2)])

Distribution: pure data-parallel over B=16 complexes, 2 per NeuronCore.

Device-side design (per core, per complex, per side):
  - H fed host-transposed [d=128, N] in fp8-e4m3 so the feature axis sits on
    SBUF partitions AND the dominant HBM stream is 1 byte/elem: the kernel is
    memory-bound, so fp8 halves the roofline vs fp16 (~1e-3 score noise).
  - tiny v_k = W_k @ hbar vectors (0.4% of FLOPs) folded into host prep with
    the 1/(N*sqrt(d)) scale and a x64 power-of-two gain so fp8 stores them in
    normal range; the exp activation applies the 1/64 back on ACT for free.
  - scores for 4 groups of 1024 columns computed concurrently via PE
    column-tiling (tile_position=(0,32g)); one wide [128, 1024] exp per pass
    amortizes ACT's 352-cycle fixed cost.
  - p = exp(s) transposed back to m-on-partitions with PE matmuls whose
    moving operand is a host-built [128, 40] row-SELECTOR (not the identity):
    the transpose emits only the 40 useful (group, keypoint) rows, k-dense,
    cutting transpose streams, DVE copy width and Y-matmul width by 3.2x.
  - per 128-column block, one column-tiled matmul against a host-prebuilt
    block-diagonal X (+ones) tile accumulates Y numerator and softmax
    denominator into a [128, 40] PSUM accumulator (4 strips x 13 rows).
  - H streams as 1 MB chunks on TWO DMA queues (sync HWDGE + gpsimd SWDGE),
    one queue per side of an interleaved side-PAIR, so delivery matches the
    interleaved consumption order and neither queue ever starves compute:
    per-queue DMA rate is descriptor-latency-bound (~150 GB/s at 8 KB
    partition lines), so both queues must run concurrently; the scalar
    engine stays free to run only the exp activations.
  - host divides by Z and runs the 16 tiny 3x3 Kabsch SVDs.
"""

from contextlib import ExitStack

import numpy as np

import concourse.bass as bass
import concourse.tile as tile
from concourse import bacc
from concourse import masks, mybir
from concourse._compat import with_exitstack
from concourse.bass_utils import run_bass_kernel_spmd

B, N, D, K = 16, 16384, 128, 10
NCORES = 8
BPC = B // NCORES          # complexes per core
CH = 1024                  # score columns per group per pass
G = 4                      # PE column-tile groups (concurrent chunks)
PASS = CH * G              # m's consumed per group-pass
NPASS = N // PASS          # group-passes per (b, side)
J = CH // 128              # 128-column transpose blocks per pass
KK = G * K                 # packed keypoint columns after selector transpose
CPS = 2                    # DMA chunks per side (1 MB each, 2 passes)
FP = mybir.dt.float16
FP8 = mybir.dt.float8e4
F32 = mybir.dt.float32
VS = 64.0                  # v pre-gain so fp8(v) stays in normal range
SCALE = 1.0 / (N * np.sqrt(D))   # mean + 1/sqrt(d), folded into v on host
XBC = NPASS * J * 13       # xb columns per (b, side)


@with_exitstack
def _body(ctx, tc, hts, xbs, vv, sel_d, out):
    nc = tc.nc

    const = ctx.enter_context(tc.tile_pool(name="const", bufs=1))
    hstream = ctx.enter_context(tc.tile_pool(name="hstream", bufs=6))
    small = ctx.enter_context(tc.tile_pool(name="small", bufs=4))
    ppool = ctx.enter_context(tc.tile_pool(name="p", bufs=4))
    ptsb = ctx.enter_context(tc.tile_pool(name="ptsb", bufs=4))
    sps = ctx.enter_context(tc.tile_pool(name="spsum", bufs=2, space="PSUM"))
    ptps = ctx.enter_context(tc.tile_pool(name="ptpsum", bufs=2, space="PSUM"))
    ynps = ctx.enter_context(tc.tile_pool(name="ynum", bufs=2, space="PSUM"))

    ident = const.tile([128, 128], FP)
    masks.make_identity(nc, ident[:])
    junk = const.tile([128, 512], FP, tag="junk")
    nc.vector.memset(junk[:], 0.25)

    # Small constants: v + sel on the scalar HWDGE queue in first-need
    # order; the per-side X tiles ride the queue whose ring is free before
    # each side's first Y matmul needs them.
    v_all = const.tile([128, BPC * 2 * 32], FP8, tag="v")
    nc.scalar.dma_start(out=v_all[:], in_=vv[:, :])
    sel = const.tile([128, KK], FP, tag="sel")
    nc.scalar.dma_start(out=sel[:], in_=sel_d[:, :])
    xb_t = []
    for sg in range(4):
        xb1 = const.tile([128, XBC], FP, tag=f"xb{sg}", name=f"xb{sg}")
        xb_t.append(xb1)

    # H stream: sides are processed as interleaved PAIRS (A,B then C,D).
    # Early pair: small chunks (fast first-arrival) on the fast-starting
    # sync/scalar HWDGE rings; late pair: whole-side 2 MB chunks (16 KB
    # partition lines -> ~230 GB/s ring rate) on the slow-starting SWDGE
    # ring.  Scalar carries side B, but only its first chunk is issued here
    # -- the rest are injected into the compute loop at points where the
    # scalar ring is already free, so exp activations never queue behind a
    # ring-wait.  Every chunk gets its own SBUF slot (no WAR stalls).
    # Whole-side 2 MB chunks (16 KB partition lines -> max ring rate, and
    # both HWDGE+SWDGE rings then split HBM evenly) in strict consumption
    # order; C's first half rides the scalar ring early so the pair-1
    # ramp-up overlaps pair-0 compute.  The tiny xb tiles also crawl on
    # the scalar ring from t=0 -- DMA issue does not block the engine, so
    # the exp activations are unaffected.
    PLAN = {0: [2, 2], 1: [2, 2], 2: [2, 2], 3: [4]}
    h_tiles = {}

    def h_dma(q, sg, c):
        plan = PLAN[sg]
        p0 = sum(plan[:c])
        np_ = plan[c]
        b, side = sg // 2, sg % 2
        hc = hstream.tile([128, np_ * PASS], FP8, tag=f"hc{np_}",
                          bufs=8, name=f"hc{sg}_{c}")
        q.dma_start(out=hc[:],
                    in_=hts[side][b, :, p0 * PASS:(p0 + np_) * PASS])
        for pp in range(p0, p0 + np_):
            h_tiles[(sg, pp)] = (hc, (pp - p0) * PASS)

    nc.scalar.dma_start(out=xb_t[0][:], in_=xbs[:, 0 * XBC:1 * XBC])
    nc.scalar.dma_start(out=xb_t[1][:], in_=xbs[:, 1 * XBC:2 * XBC])
    h_dma(nc.sync, 0, 0)       # A first half: early pair in 1 MB halves
    h_dma(nc.gpsimd, 1, 0)     # B first half
    h_dma(nc.scalar, 2, 0)     # C first half (1 MB), early on scalar
    nc.scalar.dma_start(out=xb_t[2][:], in_=xbs[:, 2 * XBC:3 * XBC])
    nc.scalar.dma_start(out=xb_t[3][:], in_=xbs[:, 3 * XBC:4 * XBC])
    h_dma(nc.sync, 0, 1)       # A second half
    h_dma(nc.gpsimd, 1, 1)     # B second half
    h_dma(nc.sync, 2, 1)       # C second half (1 MB) after A
    h_dma(nc.gpsimd, 3, 0)     # D: one 2 MB chunk after B

    def do_scores(v, hc, off):
        """One pass of score matmuls + one wide exp; returns p_sb [128, CH]."""
        s_ps = sps.tile([128, CH], F32)
        for h in range(CH // 512):
            for g in range(G):   # h-outer: the 4 groups run concurrently
                nc.tensor.matmul(
                    s_ps[32 * g:32 * (g + 1), 512 * h:512 * (h + 1)],
                    v[:], hc[:, off + g * CH + 512 * h:
                             off + g * CH + 512 * (h + 1)],
                    start=True, stop=True, tile_position=(0, 32 * g))
        p_sb = ppool.tile([128, CH], FP)
        nc.scalar.activation(p_sb[:], s_ps[:],
                             mybir.ActivationFunctionType.Exp, scale=1.0 / VS)
        return p_sb

    def do_transp(p_sb):
        """Selector transposes: pt[f, KK*j + 10g+k] = p[32g+k, 128j+f]."""
        pt_ps = ptps.tile([128, J * KK], F32)
        for j in range(J):
            nc.tensor.matmul(pt_ps[:, KK * j:KK * (j + 1)],
                             p_sb[:, 128 * j:128 * (j + 1)],
                             sel[:], start=True, stop=True)
        pt = ptsb.tile([128, J * KK], FP)
        nc.vector.tensor_copy(pt[:], pt_ps[:])
        return pt

    def do_y(pt, p, yn, xb):
        # Column-tiled concurrent matmuls: block j accumulates into
        # partition strip 32*(j%4) (13 useful rows) of the [128, KK] acc.
        for j in range(J):
            nc.tensor.matmul(
                yn[32 * (j % 4):32 * (j % 4) + 13, :],
                xb[:, (p * J + j) * 13:(p * J + j + 1) * 13],
                pt[:, KK * j:KK * (j + 1)],
                start=(p == 0 and j < 4),
                stop=(p == NPASS - 1 and j >= J - 4),
                tile_position=(0, 32 * (j % 4)))

    # PE warm-up on the device-built identity (no DMA dependency): ~6us of
    # back-to-back matmuls so HAM un-throttles the PE clock (1.2 -> 2.4 GHz)
    # by the time the first H chunk lands.  The first real Y accumulation
    # overwrites this tile via its start=True flag.
    warm_ps = ynps.tile([128, 512], F32, tag="yn")
    for _ in range(45):
        nc.tensor.matmul(warm_ps[:], ident[:], junk[:], start=True, stop=True)

    def side_state(sg):
        b, side = sg // 2, sg % 2
        return {
            "v": v_all[:, sg * 32:sg * 32 + 32],
            "xb": xb_t[sg][:],
            "yn": ynps.tile([128, KK], F32, name=f"yn{sg}", tag="yn"),
            "pend_e": None, "pend_y": None, "sg": sg,
        }

    def step(st, p):
        """Advance one side's 2-deep pipeline by one pass (p=None: drain)."""
        if p is not None:
            hc, off = h_tiles[(st["sg"], p)]
            p_sb = do_scores(st["v"], hc, off)
        if st["pend_e"] is not None:
            pt = do_transp(st["pend_e"][0])
            if st["pend_y"] is not None:
                do_y(*st["pend_y"], st["yn"], st["xb"])
            st["pend_y"] = (pt, st["pend_e"][1])
            st["pend_e"] = None
        if p is not None:
            st["pend_e"] = (p_sb, p)
        elif st["pend_y"] is not None:
            do_y(*st["pend_y"], st["yn"], st["xb"])
            st["pend_y"] = None

    def finish(st):
        yn_sb = small.tile([128, KK], F32, tag="yn_sb")
        nc.vector.tensor_copy(yn_sb[:], st["yn"][:])
        b, side = st["sg"] // 2, st["sg"] % 2
        nc.sync.dma_start(out=out[b, side], in_=yn_sb[:])

    # Flat schedule: the early pair's drain steps (which wait on its last
    # exps) are interleaved with the late pair's first passes, whose chunks
    # are already resident -- PE never idles across the pair boundary.
    sa, sb, sc_, sd = [side_state(sg) for sg in range(4)]
    for p in range(NPASS):
        step(sa, p)
        step(sb, p)
    step(sc_, 0)
    step(sd, 0)
    step(sa, None)
    step(sb, None)
    step(sc_, 1)
    step(sd, 1)
    step(sa, None)
    finish(sa)
    step(sb, None)
    finish(sb)
    for p in range(2, NPASS):
        step(sc_, p)
        step(sd, p)
    step(sc_, None)
    step(sd, None)
    step(sc_, None)
    finish(sc_)
    step(sd, None)
    finish(sd)


_NC_CACHE = {}


def _build_nc():
    if "nc" in _NC_CACHE:
        return _NC_CACHE["nc"]
    nc = bacc.Bacc(None)
    h1t = nc.declare_dram_parameter("h1t", [BPC, D, N], FP8, isOutput=False)
    h2t = nc.declare_dram_parameter("h2t", [BPC, D, N], FP8, isOutput=False)
    xb = nc.declare_dram_parameter("xb", [128, BPC * 2 * XBC], FP,
                                   isOutput=False)
    vv = nc.declare_dram_parameter("vv", [128, BPC * 2 * 32], FP8,
                                   isOutput=False)
    sel = nc.declare_dram_parameter("sel", [128, KK], FP, isOutput=False)
    out = nc.declare_dram_parameter("out", [BPC, 2, 128, KK], F32,
                                    isOutput=True)
    with tile.TileContext(nc) as tc:
        _body(tc, (h1t, h2t), xb, vv, sel, out)
    nc.compile()
    _NC_CACHE["nc"] = nc
    return nc


def _make_xblk(X):
    """X [B, N, 3] f32 -> [B, 128, XBC] fp16 block-diagonal layout.

    Column (p*J+j)*13 + 3g+c at partition f holds X[b, (p*G+g)*CH + j*128 + f, c];
    column (p*J+j)*13 + 12 is 1.0 (softmax-denominator ones column).
    """
    Bn = X.shape[0]
    Xr = X.reshape(Bn, NPASS, G, J, 128, 3).transpose(0, 1, 3, 4, 2, 5)
    Xb = Xr.reshape(Bn, NPASS, J, 128, 12)
    ones = np.ones((Bn, NPASS, J, 128, 1), np.float32)
    full = np.concatenate([Xb, ones], -1)            # [B, NPASS, J, 128, 13]
    return np.ascontiguousarray(
        full.transpose(0, 3, 1, 2, 4).reshape(Bn, 128, XBC)
    ).astype(np.float16)


def _prep(H1, H2, X1, X2, W1, W2):
    f8 = mybir.dt.np(FP8)
    h1T = np.ascontiguousarray(H1.transpose(0, 2, 1)).astype(f8)
    h2T = np.ascontiguousarray(H2.transpose(0, 2, 1)).astype(f8)
    xb = np.stack([_make_xblk(X1), _make_xblk(X2)], axis=1)  # [B,2,128,XBC]
    # v[b, 0] = W1 @ scaled-mean(H2[b]) drives the H1-side scores;
    # v[b, 1] = W2 @ scaled-mean(H1[b]).  x64 keeps fp8 in normal range;
    # the exp activation divides it back out.
    vv = np.zeros((B, 2, 128, 32), np.float32)
    vv[:, 0, :, :K] = np.einsum('kde,be->bdk', W1, H2.sum(1)) * (SCALE * VS)
    vv[:, 1, :, :K] = np.einsum('kde,be->bdk', W2, H1.sum(1)) * (SCALE * VS)
    vv = vv.astype(f8)
    # Row selector: sel[32g+k, 10g+k] = 1 picks the 40 useful score rows.
    sel = np.zeros((128, KK), np.float16)
    for g in range(G):
        for k in range(K):
            sel[32 * g + k, K * g + k] = 1.0
    in_maps = []
    for c in range(NCORES):
        s = slice(c * BPC, (c + 1) * BPC)
        xbc = xb[s].transpose(2, 0, 1, 3).reshape(128, -1)
        vvc = vv[s].transpose(2, 0, 1, 3).reshape(128, -1)
        in_maps.append({
            "h1t": h1T[s], "h2t": h2T[s],
            "xb": np.ascontiguousarray(xbc),
            "vv": np.ascontiguousarray(vvc),
            "sel": sel,
        })
    return in_maps


def _kabsch_np(P, Q):
    c1 = P.mean(0)
    c2 = Q.mean(0)
    Hm = (P - c1).T @ (Q - c2)
    U, _, Vt = np.linalg.svd(Hm)
    sign = np.sign(np.linalg.det(U @ Vt))
    R = U @ np.diag([1.0, 1.0, sign]) @ Vt
    t = c2 - c1 @ R
    return P @ R + t


def _finalize(res):
    Y = np.zeros((B, 2, K, 3), np.float32)
    for c in range(NCORES):
        yn = np.asarray(res[c]["out"], np.float32)  # [BPC, 2, 128, KK]
        for bl in range(BPC):
            for side in range(2):
                acc = yn[bl, side]
                Ynum = np.zeros((K, 3), np.float32)
                Z = np.zeros(K, np.float32)
                for s in range(4):
                    for g in range(G):
                        Z += acc[32 * s + 12, K * g:K * g + K]
                        Ynum += acc[32 * s + 3 * g:32 * s + 3 * g + 3,
                                    K * g:K * g + K].T
                Y[c * BPC + bl, side] = Ynum / Z[:, None]
    Y1, Y2 = Y[:, 0], Y[:, 1]
    Y1a = np.stack([
        _kabsch_np(Y1[b].astype(np.float64), Y2[b].astype(np.float64))
        for b in range(B)
    ]).astype(np.float32)
    return np.stack([Y1, Y2, Y1a], axis=1)


def kernel(H1, H2, X1, X2, W1, W2):
    args = [np.asarray(a, np.float32) for a in (H1, H2, X1, X2, W1, W2)]
    in_maps = _prep(*args)
    nc = _build_nc()
    res = run_bass_kernel_spmd(nc, in_maps, list(range(NCORES))).results
    return _finalize(res)
